# revision 56
# baseline (speedup 1.0000x reference)
"""Trainium2 Bass kernel for a 3-type heterogeneous GraphSAGE GNN.

Full-input contract: kernel(**inputs) takes the unsharded numpy inputs and
returns the full [300000, 2] output. Internally (SPMD over 8 cores):
  - Nodes are relabeled so each core owns a contiguous local range of
    37632 padded nodes (12544 per type); edges are sharded by dst owner.
    The replicated x_full (xf) uses a piece-major global numbering so each
    AllGather piece lands contiguously and gates only the src blocks it
    covers.
  - xf / x_own store compact bf16 features as node-PAIR rows
    [n_nodes/2, 128]: node n at row n//2, col half (n%2)*64. This halves
    AllGather bytes vs f32 while keeping the 256B gather-element
    granularity the SWDGE path requires; each gather descriptor fetches
    one pair row and per-chunk even/odd-parity one-hots route the right
    half into the aggregation matmuls.
  - Per core, edges are sorted by (pass, src block of 32768, dst group,
    src) and padded into a uniform schedule of 128-edge chunks (one NEFF,
    per-core data). gpsimd.dma_gather pulls pair rows in 1024-index calls
    over 4 SWDGE queues; idx/dstrel DMAs are batched in 64-chunk strips.
  - Segment-sum: bf16 one-hots [128 edges x 128 dsts] built on DVE via
    is_equal(dstrel_even/odd, iota) feed two accumulating PE matmuls per
    chunk (even half, odd half of the gathered pair) into a 64-slot f32
    PSUM ring; banks flush into the SBUF accumulator agg; mean = agg *
    winv per post group. bf16 operands keep PE at 1 cycle/row and avoid
    the f32 2-pass LDWEIGHTS split.
  - The encoder consumes a host-transposed xrawT [49, npc] bf16 (ones row
    folds the bias); the x@W1r term of layer 1 is folded into the encoder
    weights, and y1@W2r + b2 is computed during post-L1 (xwr buffer).
    The classifier runs transposed and emits out^T [2, npc]; the host
    untransposes and un-permutes.
  - A per-layer probe DMA + Pool gate op data-depends on the layer's
    AllGather pieces before the first gather of that layer.
"""

import numpy as np
import ml_dtypes

import bass_rust
import concourse.bass as bass
import concourse.bacc as bacc
import concourse.mybir as mybir
import concourse.tile as tile
from concourse.masks import make_identity
from concourse.bass_utils import run_bass_kernel_spmd

F32 = mybir.dt.float32
BF16 = mybir.dt.bfloat16
I16 = mybir.dt.int16
BF = ml_dtypes.bfloat16

FULL_CFG = dict(type_size=100000, E=4800000, cores=8, blk=65536,
                strip_chunks=64, oh_chunks=16, gather_chunks=8,
                ag_pieces=3, layer_splits=2, split_tail_blocks=2,
                swdge_queues=4, h=64)


def derive(cfg):
    cores = cfg["cores"]
    seg = cfg["type_size"] // cores          # real nodes per (core, type)
    assert seg * cores == cfg["type_size"]
    segp = -(-seg // 128) * 128              # padded to tile multiple
    npc = 3 * segp                           # nodes per core (padded)
    nptot = cores * npc
    tiles = npc // 128
    groups = tiles                           # 128-dst groups per core
    nblk = -(-nptot // cfg["blk"])
    d = dict(cfg)
    d.update(seg=seg, segp=segp, npc=npc, nptot=nptot, tiles=tiles,
             groups=groups, nblk=nblk)
    return d


def node_perm(d):
    """perm_of_orig[j] = per-core-local padded id of original node j
    (core-major: core * npc + t * segp + local)."""
    ts, seg, segp, npc = d["type_size"], d["seg"], d["segp"], d["npc"]
    j = np.arange(ts)
    core = j // seg
    local = j % seg
    parts = [core * npc + t * segp + local for t in range(3)]
    return np.concatenate(parts)


def glob_of_loc(d):
    """Map core-major local id -> piece-major xf row (AllGather pieces:
    xf = [piece s: core0 slice s, core1 slice s, ...])."""
    npc, nptot, cores = d["npc"], d["nptot"], d["cores"]
    P = d["ag_pieces"]
    psz = npc // P
    assert psz * P == npc
    lid = np.arange(nptot)
    c = lid // npc
    pos = lid % npc
    s = pos // psz
    return s * (cores * psz) + c * psz + pos % psz


class Sched:
    pass


def plan(d, edge_index):
    """Build the uniform schedule + per-core edge data arrays."""
    cores, npc, nptot, blk = d["cores"], d["npc"], d["nptot"], d["blk"]
    groups, nblk, sc = d["groups"], d["nblk"], d["strip_chunks"]
    ohc = d["oh_chunks"]
    gc = d["gather_chunks"]
    nsp = d["layer_splits"]
    gps = groups // nsp
    assert gps * nsp == groups
    FB = nblk - d["split_tail_blocks"]

    perm = node_perm(d)
    g_of_l = glob_of_loc(d)
    src_p = g_of_l[perm[np.asarray(edge_index[0], dtype=np.int64)]]
    dst_p = perm[np.asarray(edge_index[1], dtype=np.int64)]

    deg = np.bincount(dst_p, minlength=nptot).astype(np.float64)
    winv_full = (1.0 / np.maximum(deg, 1.0)).astype(np.float32)

    # per-core sorted edge arrays + per-(block, group) counts
    core_of = dst_p // npc
    per_core = []
    counts = np.zeros((cores, nblk, groups), np.int64)
    for c in range(cores):
        m = core_of == c
        es = src_p[m]
        ed = dst_p[m] - c * npc
        b = es // blk
        g = ed // 128
        # cells contiguous by pass; within cell sort by src
        pr = np.where(b < FB, b, FB + (g // gps) * (nblk - FB) + (b - FB))
        order = np.lexsort((es, g, pr))
        es, ed, b, g = es[order], ed[order], b[order], g[order]
        np.add.at(counts[c], (b, g), 1)
        per_core.append((es, ed, b, g))

    nch = np.maximum(1, -(-counts.max(axis=0) // 128))  # [nblk, groups]

    # cell order: blocks 0..FB-1 run all groups (block-major, matching
    # AllGather piece arrival); the last split_tail_blocks blocks run in
    # group-split passes so the first gps groups of agg complete early,
    # letting post + the next AllGather piece overlap the layer tail
    passes = [(b, 0, groups) for b in range(FB)]
    for sp in range(nsp):
        for b in range(FB, nblk):
            passes.append((b, sp * gps, (sp + 1) * gps))
    cell_b_arr, cell_g_arr, cell_g0 = [], [], []
    for (b, g0, g1) in passes:
        for g in range(g0, g1):
            cell_b_arr.append(b)
            cell_g_arr.append(g)
            cell_g0.append(g0)
    cell_b_arr = np.array(cell_b_arr)
    cell_g_arr = np.array(cell_g_arr)
    cell_g0 = np.array(cell_g0)
    cell_pass = np.concatenate(
        [np.full(g1 - g0, pi) for pi, (b, g0, g1) in enumerate(passes)])
    cell_g1 = np.array([passes[p][2] for p in cell_pass])
    nch_ord = nch[cell_b_arr, cell_g_arr]

    chunk_cell = np.repeat(np.arange(len(cell_b_arr)), nch_ord)
    chunk_b = cell_b_arr[chunk_cell]
    chunk_g = cell_g_arr[chunk_cell]
    chunk_g0 = cell_g0[chunk_cell]
    chunk_g1 = cell_g1[chunk_cell]
    chunk_pass = cell_pass[chunk_cell]
    nchunks = len(chunk_b)
    first_of_cell = np.concatenate([[0], np.cumsum(nch_ord)])[:-1]
    kk = np.arange(nchunks) - first_of_cell[chunk_cell]
    chunk_start = kk == 0
    chunk_stop = kk == nch_ord[chunk_cell] - 1

    # strips: cut chunk list per pass at strip_chunks boundary
    strips = []  # (b, c0, n, idx_col_off)
    idx_off = 0
    i = 0
    while i < nchunks:
        j = i
        while (j < nchunks and chunk_pass[j] == chunk_pass[i]
               and j - i < sc):
            j += 1
        strips.append((int(chunk_b[i]), i, j - i, idx_off))
        idx_off += (j - i) * 8
        i = j
    idx_cols = idx_off
    strip_of_chunk = np.zeros(nchunks, np.int64)
    strip_c0 = np.zeros(nchunks, np.int64)
    for si, (b, c0, n, io) in enumerate(strips):
        strip_of_chunk[c0:c0 + n] = si
        strip_c0[c0:c0 + n] = c0

    # op lists (two parts: h0 = full blocks + first tail split; h1 = rest)
    npass_h0 = FB + (nblk - FB)
    ops_parts = [[], []]
    for si, (b, c0, n, io) in enumerate(strips):
        ops = ops_parts[0 if chunk_pass[c0] < npass_h0 else 1]
        ops.append(("strip", si))
        for k in range(n):
            if k % gc == 0:
                ops.append(("gather", si, k, min(gc, n - k)))
            if k % ohc == 0:
                ops.append(("oh", si, k, min(ohc, n - k)))
            ci = c0 + k
            g = int(chunk_g[ci])
            g0 = int(chunk_g0[ci])
            g_rel = g - g0
            ops.append(("mm", si, k, g_rel,
                        bool(chunk_start[ci]), bool(chunk_stop[ci])))
            nxt = ci + 1
            if chunk_stop[ci]:
                bank_end = (nxt == nchunks
                            or chunk_pass[nxt] != chunk_pass[ci]
                            or (chunk_g[nxt] - g0) // 8 != g_rel // 8)
                if bank_end:
                    r_lo = (g_rel // 8) * 8
                    g_lo = g0 + r_lo
                    g_hi = min(g_lo + 7, int(chunk_g1[ci]) - 1)
                    ops.append(("flush", g_lo, g_hi, r_lo))

    # ---- per-core data arrays ----
    # padded slots: each ordered cell occupies nch*128 consecutive slots
    cell_nslots = nch_ord * 128
    cell_pad_start = np.concatenate([[0], np.cumsum(cell_nslots)])
    total_slots = int(cell_pad_start[-1])
    assert total_slots == nchunks * 128
    slot = np.arange(total_slots)
    cell_of_slot = np.searchsorted(cell_pad_start, slot, "right") - 1
    within = slot - cell_pad_start[cell_of_slot]

    # chunk index of each slot & strip-local edge index
    ch_of_slot = slot // 128
    strip_local = (ch_of_slot - strip_c0[ch_of_slot]) * 128 + slot % 128
    idx_col = np.array([strips[s][3] for s in strip_of_chunk[ch_of_slot]]) \
        + strip_local // 16
    idx_row = strip_local % 16
    slot_b = chunk_b[ch_of_slot]
    slot_g = chunk_g[ch_of_slot]

    # per-chunk even/odd dstrel column layout: strip si (chunks c0..c0+n)
    # occupies dstrel cols [2*c0, 2*c0+2n): even-parity block then odd.
    ch_ids = np.arange(nchunks)
    col_e = 2 * strip_c0[ch_ids] + (ch_ids - strip_c0[ch_ids])
    col_o = col_e + np.array([strips[s_][2] for s_ in strip_of_chunk])

    idx_all = np.zeros((cores, 128, idx_cols), np.int16)
    dstrel_all = np.full((cores, 128, 2 * nchunks), -1.0, np.float32)
    for c in range(cores):
        es, ed, b, g = per_core[c]
        ccounts = counts[c][cell_b_arr, cell_g_arr]
        cell_start = np.concatenate([[0], np.cumsum(ccounts)])
        real = within < ccounts[cell_of_slot]
        src_idx = cell_start[cell_of_slot] + np.minimum(
            within, np.maximum(ccounts[cell_of_slot] - 1, 0))
        esv = np.where(real, es[np.minimum(src_idx, len(es) - 1)]
                       if len(es) else 0, 0)
        edv = np.where(real, ed[np.minimum(src_idx, len(ed) - 1)]
                       if len(ed) else 0, -1)
        rel = np.where(real, esv - slot_b * blk, 0).astype(np.int64)
        assert rel.min() >= 0 and rel.max() < blk
        drel = np.where(real, edv - slot_g * 128, -1.0).astype(np.float32)
        # pair-row index (each 256B gather elem covers nodes 2j, 2j+1)
        for r in range(8):
            idx_all[c, idx_row + 16 * r, idx_col] = (rel >> 1).astype(np.int16)
        par = (esv % 2).astype(np.int64)
        cols = np.where(par == 0, col_e[ch_of_slot], col_o[ch_of_slot])
        dstrel_all[c, slot % 128, cols] = drel

    s = Sched()
    s.d = d
    s.perm = perm
    s.strips = strips
    s.ops_parts = ops_parts
    s.gps = gps
    s.nchunks = nchunks
    s.idx_cols = idx_cols
    s.winv_full = winv_full
    s.idx_all = idx_all
    s.dstrel_all = dstrel_all
    return s


def core_inputs(s, x_individual, x_company, x_trust,
                W_ind, b_ind, W_com, b_com, W_tru, b_tru,
                W1l, W1r, b1, W2l, W2r, b2, Wc1, bc1, Wc2, bc2):
    d = s.d
    cores, seg, segp, npc, groups = \
        d["cores"], d["seg"], d["segp"], d["npc"], d["groups"]
    raws = [np.asarray(x_individual, np.float32),
            np.asarray(x_company, np.float32),
            np.asarray(x_trust, np.float32)]
    Ws = [np.asarray(W_ind, np.float32), np.asarray(W_com, np.float32),
          np.asarray(W_tru, np.float32)]
    bs = [np.asarray(b_ind, np.float32), np.asarray(b_com, np.float32),
          np.asarray(b_tru, np.float32)]
    h = d["h"]
    kenc = 49  # 48 padded features + ones row

    W1l = np.asarray(W1l, np.float32)
    W1r = np.asarray(W1r, np.float32)
    b1 = np.asarray(b1, np.float32)
    W2l = np.asarray(W2l, np.float32)
    W2r = np.asarray(W2r, np.float32)
    b2 = np.asarray(b2, np.float32)
    Wc1 = np.asarray(Wc1, np.float32)
    bc1 = np.asarray(bc1, np.float32)
    Wc2 = np.asarray(Wc2, np.float32)
    bc2 = np.asarray(bc2, np.float32)

    shared = {}
    for t in range(3):
        wx = np.zeros((kenc, h), np.float32)
        wx[:Ws[t].shape[0], :] = Ws[t]
        wx[48, :] = bs[t]
        shared[f"w_enc_x{t}"] = wx.astype(BF)
        # folded x@W1r path: (x_enc) @ W1r + b1, from raw features
        wr = np.zeros((kenc, h), np.float32)
        wr[:Ws[t].shape[0], :] = Ws[t] @ W1r
        wr[48, :] = bs[t] @ W1r + b1
        shared[f"w_enc_r{t}"] = wr.astype(BF)

    w2r_ext = np.zeros((h + 1, h), np.float32)
    w2r_ext[:h, :] = W2r
    w2r_ext[h, :] = b2
    wc1_ext = np.zeros((h + 1, 32), np.float32)
    wc1_ext[:h, :] = Wc1
    wc1_ext[h, :] = bc1
    wc2_ext = np.zeros((33, 2), np.float32)
    wc2_ext[:32, :] = Wc2
    wc2_ext[32, :] = bc2

    shared.update({
        "w1l": W1l.astype(BF), "w2l": W2l.astype(BF),
        "w2r_ext": w2r_ext.astype(BF),
        "wc1_ext": wc1_ext.astype(BF), "wc2_ext": wc2_ext.astype(BF),
        "iota_rep": np.tile(np.arange(128, dtype=np.float32)[None, :],
                            (128, 1)).astype(BF),
    })

    in_maps = []
    for c in range(cores):
        xrT = np.zeros((kenc, npc), np.float32)
        for t in range(3):
            r0 = t * segp
            xrT[:raws[t].shape[1], r0:r0 + seg] = \
                raws[t][c * seg:(c + 1) * seg].T
            xrT[48, r0:r0 + seg] = 1.0
        winv = s.winv_full[c * npc:(c + 1) * npc] \
            .reshape(groups, 128).T.copy()
        m = dict(shared)
        m.update(xrawT=xrT.astype(BF), idx=s.idx_all[c],
                 dstrel=s.dstrel_all[c].astype(BF), winv=winv)
        in_maps.append(m)
    return in_maps


def build_program(s, skip=()):
    skip = set(skip)
    d = s.d
    cores, npc, nptot, blk = d["cores"], d["npc"], d["nptot"], d["blk"]
    tiles, groups, nblk, h = d["tiles"], d["groups"], d["nblk"], d["h"]
    sc, ohc = d["strip_chunks"], d["oh_chunks"]
    slots = 64
    kenc = 49
    GT = 8                                   # tiles per post/encoder group
    ngroups = -(-tiles // GT)
    seg_tiles = d["segp"] // 128

    nc = bacc.Bacc("TRN2", target_bir_lowering=False, debug=False,
                   num_devices=cores,
                   num_swdge_queues=d.get("swdge_queues", 1),
                   dynamic_dma_scratch_size=d.get("dma_scratch", 16384))

    di = {}
    def inp(name, shape, dt=F32):
        di[name] = nc.dram_tensor(name, list(shape), dt, kind="ExternalInput")
        return di[name]

    inp("xrawT", [kenc, npc], BF16)
    if "nocoll" in skip:
        inp("xf0_in", [nptot // 2, 2 * h], BF16)
        inp("xf1_in", [nptot // 2, 2 * h], BF16)
    inp("idx", [128, s.idx_cols], I16)
    inp("dstrel", [128, 2 * s.nchunks], BF16)
    inp("winv", [128, groups])
    inp("iota_rep", [128, 128], BF16)
    for t in range(3):
        inp(f"w_enc_x{t}", [kenc, h], BF16)
        inp(f"w_enc_r{t}", [kenc, h], BF16)
    inp("w1l", [h, h], BF16)
    inp("w2l", [h, h], BF16)
    inp("w2r_ext", [h + 1, h], BF16)
    inp("wc1_ext", [h + 1, 32], BF16)
    inp("wc2_ext", [33, 2], BF16)
    out_d = nc.dram_tensor("out", [2, npc], F32, kind="ExternalOutput")

    AG = "AllGather"
    ADD = mybir.AluOpType.add
    MUL = mybir.AluOpType.mult
    EQ = mybir.AluOpType.is_equal
    BYP = mybir.AluOpType.bypass
    RELU = mybir.ActivationFunctionType.Relu

    with tile.TileContext(nc) as tc:
        with tc.tile_pool(name="persist", bufs=1) as pp, \
             tc.tile_pool(name="dram", bufs=1, space="DRAM") as dramp:
            # constants to SBUF
            def csb(name, shape, dt=F32):
                t_ = pp.tile(list(shape), dt, tag=name)
                nc.sync.dma_start(t_[:], di[name].ap())
                return t_
            iota_sb = csb("iota_rep", [128, 128], BF16)
            winv_sb = csb("winv", [128, groups])
            wex_sb = [csb(f"w_enc_x{t}", [kenc, h], BF16) for t in range(3)]
            wer_sb = [csb(f"w_enc_r{t}", [kenc, h], BF16) for t in range(3)]
            w1l_sb = csb("w1l", [h, h], BF16)
            w2l_sb = csb("w2l", [h, h], BF16)
            w2r_sb = csb("w2r_ext", [h + 1, h], BF16)
            wc1_sb = csb("wc1_ext", [h + 1, 32], BF16)
            wc2_sb = csb("wc2_ext", [33, 2], BF16)
            ident = pp.tile([128, 128], F32, tag="ident")
            make_identity(nc, ident[:])
            identb = pp.tile([128, 128], BF16, tag="identb")
            nc.scalar.copy(identb[:], ident[:])

            # AG-completion sems: gathers of layer L wait for all of layer
            # L's AllGather pieces before issuing, so collective traffic
            # never contends with gather descriptor streams. Cleared at
            # start (sems persist across NEFF re-runs).
            agp_sb = [pp.tile([1, 2 * h], BF16, tag="agp0", name="agp0"),
                      pp.tile([1, 2 * h], BF16, tag="agp1", name="agp1")]
            gate_sb = pp.tile([1, 2 * h], BF16, tag="gate")

            def ag_probe(L, end_row):
                # 1-row read of the piece tail; Tile orders it after the
                # collective's output write.
                nc.sync.dma_start(agp_sb[L][:],
                                  xf[L][end_row - 1:end_row, :])

            agg = pp.tile([128, groups * h], F32, tag="agg")
            xwr = pp.tile([128, groups * h], BF16, tag="xwr")

            # x_own/xf hold compact bf16 features as node-PAIR rows
            # [n_nodes/2, 2h]: node n lives at row n//2, col half (n%2)*h.
            # Each 256B gather elem fetches one pair row; per chunk the
            # even/odd-parity one-hots route the right half into agg.
            x_own0 = dramp.tile([npc // 2, 2 * h], BF16)
            x_own1 = dramp.tile([npc // 2, 2 * h], BF16)
            xf = [dramp.tile([nptot // 2, 2 * h], BF16, name="xf0"),
                  dramp.tile([nptot // 2, 2 * h], BF16, name="xf1")]
            x_own = [x_own0, x_own1]
            if "nocoll" in skip:
                xf_slice = lambda L, a, b: \
                    di[f"xf{L}_in"].ap()[a // 2:b // 2, :]
            else:
                xf_slice = lambda L, a, b: xf[L][a // 2:b // 2, :]

            def pair_write_ap(own, t0, gt):
                # dest AP for nodes [t0*128, (t0+gt)*128) in pair-row
                # layout; flat elem offset of node n is n*h, so the
                # (p, t, f) iteration is affine: [[h,128],[128*h,gt],[1,h]].
                a = own[:, :].copy()
                a.ap = bass_rust.VecI64Pair([[h, 128], [128 * h, gt], [1, h]])
                a.offset = t0 * 128 * h
                return a

            # ---------------- encoder ----------------
            with tc.tile_pool(name="encio", bufs=1) as pio, \
                 tc.tile_pool(name="enc", bufs=2) as pe, \
                 tc.tile_pool(name="encps", bufs=2, space="PSUM") as pse:
                xrT = pio.tile([kenc, tiles * 128], BF16)
                nc.sync.dma_start(xrT[:], di["xrawT"].ap())
                GT_E = 16
                for gi in range(0 if "enc" in skip else -(-tiles // GT_E)):
                    t0 = gi * GT_E
                    gt = min(GT_E, tiles - t0)
                    ps_x = pse.tile([128, GT_E * h], F32, tag="psx")
                    ps_w = pse.tile([128, GT_E * h], F32, tag="psw")
                    for k in range(gt):
                        t = t0 + k
                        ty = t // seg_tiles
                        lhs = xrT[:, t * 128:(t + 1) * 128]
                        nc.tensor.matmul(out=ps_x[:, k * h:(k + 1) * h],
                                         lhsT=lhs, rhs=wex_sb[ty][:],
                                         start=True, stop=True)
                        nc.tensor.matmul(out=ps_w[:, k * h:(k + 1) * h],
                                         lhsT=lhs, rhs=wer_sb[ty][:],
                                         start=True, stop=True)
                    x0s = pe.tile([128, GT_E * h], BF16, tag="x0s")
                    nc.scalar.copy(x0s[:, :gt * h], ps_x[:, :gt * h])
                    nc.sync.dma_start(
                        pair_write_ap(x_own0, t0, gt),
                        x0s[:, :gt * h].rearrange("p (t f) -> p t f", f=h))
                    nc.scalar.copy(xwr[:, t0 * h:(t0 + gt) * h],
                                   ps_w[:, :gt * h])
            P0 = d["ag_pieces"]
            psz0 = npc // P0
            for sp in range(P0 if "nocoll" not in skip else 0):
                nc.gpsimd.collective_compute(
                    AG, BYP, replica_groups=[list(range(cores))],
                    ins=[x_own0[sp * psz0 // 2:(sp + 1) * psz0 // 2, :]],
                    outs=[xf[0][sp * cores * psz0 // 2:
                                (sp + 1) * cores * psz0 // 2, :]])
                ag_probe(0, (sp + 1) * cores * psz0 // 2)

            # -------- SAGE layers (post/cls interleaved at h0/h1) -------
            GT_C = 8
            ngroups_c = -(-tiles // GT_C)
            gps = s.gps
            gi_h0 = gps // GT          # post groups fully inside h0
            gi_h0_c = gps // GT_C
            P = d["ag_pieces"]
            psz = npc // P

            def emit_ops(ops, pa, po, psum_agg, L, slots, wait=None):
                if "sage" in skip:
                    return
                gate = wait if "nocoll" not in skip else None
                cur = {}
                ohmod = d.get("oh_pool_mod", 0)
                ohi = 0
                for op in ops:
                    if op[0] == "strip":
                        si = op[1]
                        b, c0, n, ioff = s.strips[si]
                        idx_sb = pa.tile([128, sc * 8], I16, tag="idx")
                        nc.sync.dma_start(
                            idx_sb[:, :n * 8],
                            di["idx"].ap()[:, ioff:ioff + n * 8])
                        dst_sb = pa.tile([128, 2 * sc], BF16, tag="dst")
                        nc.sync.dma_start(
                            dst_sb[:, :2 * n],
                            di["dstrel"].ap()[:, 2 * c0:2 * c0 + 2 * n])
                        msgs = pa.tile([128, sc * 2 * h], BF16, tag="msgs")
                        if gate is not None:
                            # Pool op whose output the first gather
                            # overwrites: WAR-chains all gathers behind this
                            # layer's AllGather pieces (via the probe tile).
                            nc.gpsimd.tensor_tensor(
                                out=msgs[0:1, 0:2 * h],
                                in0=agp_sb[gate][:],
                                in1=agp_sb[gate][:], op=ADD)
                            gate = None
                        rows = min(blk, nptot - b * blk)
                        cur = dict(msgs=msgs, dst=dst_sb, idx=idx_sb,
                                   b=b, rows=rows, n=n)
                    elif op[0] == "gather":
                        _, si, k0, m = op
                        b = cur["b"]
                        rows = cur["rows"]
                        nq = d.get("swdge_queues", 1)
                        nc.gpsimd.dma_gather(
                            out_ap=cur["msgs"][:, k0 * 2 * h:(k0 + m) * 2 * h]
                            .rearrange("p (c f) -> p c f", f=2 * h),
                            in_ap=xf_slice(L, b * blk, b * blk + rows),
                            idxs_ap=cur["idx"][:, k0 * 8:(k0 + m) * 8],
                            num_idxs=m * 128, num_idxs_reg=m * 128,
                            elem_size=2 * h,
                            queue_num=(k0 // 8) % nq)
                    elif op[0] == "oh":
                        _, si, k0, m = op
                        n = cur["n"]
                        oh = po.tile([128, 2 * ohc * 128], BF16, tag="oh")
                        ohi += 1
                        eng = (nc.gpsimd if ohmod and ohi % ohmod == 0
                               else nc.vector)
                        for half, cofs in ((0, k0), (1, n + k0)):
                            eng.tensor_tensor(
                                out=oh[:, half * ohc * 128:
                                       half * ohc * 128 + m * 128]
                                .rearrange("p (c w) -> p c w", w=128),
                                in0=cur["dst"][:, cofs:cofs + m][:, :, None]
                                .to_broadcast([128, m, 128]),
                                in1=iota_sb[:][:, None, :]
                                .to_broadcast([128, m, 128]),
                                op=EQ)
                        cur["oh"] = oh
                        cur["k0"] = k0
                    elif op[0] == "mm":
                        _, si, k, g_rel, st, sp = op
                        sl = g_rel % slots
                        ko = k - cur["k0"]
                        nc.tensor.matmul(
                            out=psum_agg[:, sl * h:(sl + 1) * h],
                            lhsT=cur["oh"][:, ko * 128:(ko + 1) * 128],
                            rhs=cur["msgs"][:, k * 2 * h:k * 2 * h + h],
                            start=st, stop=False)
                        nc.tensor.matmul(
                            out=psum_agg[:, sl * h:(sl + 1) * h],
                            lhsT=cur["oh"][:, ohc * 128 + ko * 128:
                                           ohc * 128 + (ko + 1) * 128],
                            rhs=cur["msgs"][:, k * 2 * h + h:
                                            (k + 1) * 2 * h],
                            start=False, stop=sp)
                    else:  # flush
                        _, g_lo, g_hi, r_lo = op
                        sl = r_lo % slots
                        w = (g_hi - g_lo + 1) * h
                        nc.vector.tensor_tensor(
                            out=agg[:, g_lo * h:g_lo * h + w],
                            in0=agg[:, g_lo * h:g_lo * h + w],
                            in1=psum_agg[:, sl * h:sl * h + w],
                            op=ADD)

            def mean_groups(gi, t0, gt):
                nc.vector.tensor_tensor(
                    out=agg[:, t0 * h:(t0 + gt) * h].rearrange(
                        "p (g f) -> p g f", f=h),
                    in0=agg[:, t0 * h:(t0 + gt) * h].rearrange(
                        "p (g f) -> p g f", f=h),
                    in1=winv_sb[:, t0:t0 + gt][:, :, None]
                    .to_broadcast([128, gt, h]),
                    op=MUL)

            def post_l1(pq, psp, gi0, gi1):
                if "post1" in skip:
                    return
                # y1 = relu(mean@W1l + xwr1); xwr2 = y1@W2r + b2
                for gi in range(gi0, gi1):
                    t0 = gi * GT
                    gt = min(GT, tiles - t0)
                    mean_groups(gi, t0, gt)
                    mb = pq.tile([128, GT * h], BF16, tag="mb")
                    nc.scalar.copy(mb[:, :gt * h],
                                   agg[:, t0 * h:(t0 + gt) * h])
                    tp = psp.tile([64, GT * 128], BF16, tag="tp")
                    for k in range(gt):
                        nc.tensor.transpose(
                            out=tp[:, k * 128:(k + 1) * 128],
                            in_=mb[:, k * h:(k + 1) * h],
                            identity=identb[:])
                    mT = pq.tile([64, GT * 128], BF16, tag="mT")
                    nc.scalar.copy(mT[:, :gt * 128], tp[:, :gt * 128])
                    ym = psp.tile([128, GT * h], F32, tag="ym")
                    for k in range(gt):
                        nc.tensor.matmul(
                            out=ym[:, k * h:(k + 1) * h],
                            lhsT=mT[:, k * 128:(k + 1) * 128],
                            rhs=w1l_sb[:], start=True, stop=True)
                    tmp = pq.tile([128, GT * h], F32, tag="tmp")
                    nc.vector.tensor_tensor(
                        out=tmp[:, :gt * h], in0=ym[:, :gt * h],
                        in1=xwr[:, t0 * h:(t0 + gt) * h], op=ADD)
                    ys = pq.tile([128, GT * h], F32, tag="ys")
                    nc.scalar.activation(out=ys[:, :gt * h],
                                         in_=tmp[:, :gt * h], func=RELU)
                    yb = pq.tile([128, GT * h], BF16, tag="yb")
                    nc.scalar.copy(yb[:, :gt * h], ys[:, :gt * h])
                    nc.sync.dma_start(
                        pair_write_ap(x_own1, t0, gt),
                        yb[:, :gt * h].rearrange("p (t f) -> p t f", f=h))
                    tp2 = psp.tile([64, GT * 128], BF16, tag="tp")
                    for k in range(gt):
                        nc.tensor.transpose(
                            out=tp2[:, k * 128:(k + 1) * 128],
                            in_=yb[:, k * h:(k + 1) * h],
                            identity=identb[:])
                    yT = pq.tile([h + 1, GT * 128], BF16, tag="yT")
                    nc.scalar.copy(yT[:h, :gt * 128], tp2[:, :gt * 128])
                    nc.vector.memset(yT[h:h + 1, :gt * 128], 1.0)
                    xw2 = psp.tile([128, GT * h], F32, tag="ym")
                    for k in range(gt):
                        nc.tensor.matmul(
                            out=xw2[:, k * h:(k + 1) * h],
                            lhsT=yT[:, k * 128:(k + 1) * 128],
                            rhs=w2r_sb[:], start=True, stop=True)
                    nc.scalar.copy(xwr[:, t0 * h:(t0 + gt) * h],
                                   xw2[:, :gt * h])

            def cls_range(pc, psc, gi0, gi1):
                if "cls" in skip:
                    return
                # y2 = relu(mean@W2l + xwr2); h = relu(Wc1^T y2T);
                # outT = Wc2^T hT  (biases folded via ones partitions)
                for gi in range(gi0, gi1):
                    t0 = gi * GT_C
                    gt = min(GT_C, tiles - t0)
                    mean_groups(gi, t0, gt)
                    mb = pc.tile([128, GT_C * h], BF16, tag="mb")
                    nc.scalar.copy(mb[:, :gt * h],
                                   agg[:, t0 * h:(t0 + gt) * h])
                    tp = psc.tile([64, GT_C * 128], BF16, tag="tp")
                    for k in range(gt):
                        nc.tensor.transpose(
                            out=tp[:, k * 128:(k + 1) * 128],
                            in_=mb[:, k * h:(k + 1) * h],
                            identity=identb[:])
                    mT = pc.tile([64, GT_C * 128], BF16, tag="mT")
                    nc.scalar.copy(mT[:, :gt * 128], tp[:, :gt * 128])
                    ym = psc.tile([128, GT_C * h], F32, tag="ym")
                    for k in range(gt):
                        nc.tensor.matmul(
                            out=ym[:, k * h:(k + 1) * h],
                            lhsT=mT[:, k * 128:(k + 1) * 128],
                            rhs=w2l_sb[:], start=True, stop=True)
                    tmp = pc.tile([128, GT_C * h], F32, tag="tmp")
                    nc.vector.tensor_tensor(
                        out=tmp[:, :gt * h], in0=ym[:, :gt * h],
                        in1=xwr[:, t0 * h:(t0 + gt) * h], op=ADD)
                    y2 = pc.tile([128, GT_C * h], BF16, tag="y2")
                    nc.scalar.activation(out=y2[:, :gt * h],
                                         in_=tmp[:, :gt * h], func=RELU)
                    tpb = psc.tile([64, GT_C * 128], BF16, tag="tp")
                    for k in range(gt):
                        nc.tensor.transpose(
                            out=tpb[:, k * 128:(k + 1) * 128],
                            in_=y2[:, k * h:(k + 1) * h],
                            identity=identb[:])
                    y2T = pc.tile([h + 1, GT_C * 128], BF16, tag="y2T")
                    nc.scalar.copy(y2T[:h, :gt * 128], tpb[:, :gt * 128])
                    nc.vector.memset(y2T[h:h + 1, :gt * 128], 1.0)
                    for half in range(2):
                        k0 = half * (GT_C // 2)
                        k1 = min(k0 + GT_C // 2, gt)
                        if k0 >= gt:
                            break
                        kw = k1 - k0
                        hps = psc.tile([32, (GT_C // 2) * 128], F32,
                                       tag="hps")
                        for k in range(k0, k1):
                            nc.tensor.matmul(
                                out=hps[:, (k - k0) * 128:
                                        (k - k0 + 1) * 128],
                                lhsT=wc1_sb[:],
                                rhs=y2T[:, k * 128:(k + 1) * 128],
                                start=True, stop=True)
                        hT = pc.tile([33, (GT_C // 2) * 128], BF16,
                                     tag="hT")
                        nc.scalar.activation(out=hT[:32, :kw * 128],
                                             in_=hps[:, :kw * 128],
                                             func=RELU)
                        nc.vector.memset(hT[32:33, :kw * 128], 1.0)
                        ops_ = psc.tile([2, (GT_C // 2) * 128], F32,
                                        tag="ops")
                        for k in range(kw):
                            nc.tensor.matmul(
                                out=ops_[:, k * 128:(k + 1) * 128],
                                lhsT=wc2_sb[:],
                                rhs=hT[:, k * 128:(k + 1) * 128],
                                start=True, stop=True)
                        outs = pc.tile([2, (GT_C // 2) * 128], F32,
                                       tag="outs")
                        nc.scalar.copy(outs[:, :kw * 128],
                                       ops_[:, :kw * 128])
                        nc.sync.dma_start(
                            out_d.ap()[:, (t0 + k0) * 128:
                                       (t0 + k1) * 128],
                            outs[:, :kw * 128])

            # ---- layer 1 ----
            with tc.tile_pool(name="sage0", bufs=3) as pa, \
                 tc.tile_pool(name="sageoh0", bufs=2) as po, \
                 tc.tile_pool(name="post0", bufs=2) as pq:
                nc.vector.memset(agg[:], 0.0)
                with tc.tile_pool(name="sageps0a", bufs=1,
                                  space="PSUM") as psa:
                    psum_agg = psa.tile([128, 64 * h], F32)
                    emit_ops(s.ops_parts[0], pa, po, psum_agg, 0, 64,
                             wait=0)
                gi_ag = -(-(psz // 128) // GT)
                with tc.tile_pool(name="post0psa", bufs=2,
                                  space="PSUM") as psp:
                    post_l1(pq, psp, 0, gi_ag)
                    if "nocoll" not in skip:
                        nc.gpsimd.collective_compute(
                            AG, BYP, replica_groups=[list(range(cores))],
                            ins=[x_own1[0:psz // 2, :]],
                            outs=[xf[1][0:cores * psz // 2, :]])
                        ag_probe(1, cores * psz // 2)
                    post_l1(pq, psp, gi_ag, gi_h0)
                with tc.tile_pool(name="sageps0b", bufs=1,
                                  space="PSUM") as psa:
                    psum_agg = psa.tile([128, 64 * h], F32)
                    emit_ops(s.ops_parts[1], pa, po, psum_agg, 0, 64)
                with tc.tile_pool(name="post0psb", bufs=2,
                                  space="PSUM") as psp:
                    post_l1(pq, psp, gi_h0, ngroups)
                for sp in range(1, P if "nocoll" not in skip else 1):
                    nc.gpsimd.collective_compute(
                        AG, BYP, replica_groups=[list(range(cores))],
                        ins=[x_own1[sp * psz // 2:(sp + 1) * psz // 2, :]],
                        outs=[xf[1][sp * cores * psz // 2:
                                    (sp + 1) * cores * psz // 2, :]])
                    ag_probe(1, (sp + 1) * cores * psz // 2)

            # ---- layer 2 + classifier ----
            with tc.tile_pool(name="sage1", bufs=3) as pa, \
                 tc.tile_pool(name="sageoh1", bufs=2) as po, \
                 tc.tile_pool(name="cls", bufs=2) as pc:
                nc.vector.memset(agg[:], 0.0)
                with tc.tile_pool(name="sageps1a", bufs=1,
                                  space="PSUM") as psa:
                    psum_agg = psa.tile([128, 64 * h], F32)
                    emit_ops(s.ops_parts[0], pa, po, psum_agg, 1, 64,
                             wait=1)
                with tc.tile_pool(name="clspsa", bufs=2,
                                  space="PSUM") as psc:
                    cls_range(pc, psc, 0, gi_h0_c)
                with tc.tile_pool(name="sageps1b", bufs=1,
                                  space="PSUM") as psa:
                    psum_agg = psa.tile([128, 64 * h], F32)
                    emit_ops(s.ops_parts[1], pa, po, psum_agg, 1, 64)
                with tc.tile_pool(name="clspsb", bufs=2,
                                  space="PSUM") as psc:
                    cls_range(pc, psc, gi_h0_c, ngroups_c)

    nc.compile()
    return nc


def run(cfg, inputs, trace=False):
    d = derive(cfg)
    s = plan(d, inputs["edge_index"])
    in_maps = core_inputs(
        s, **{k: v for k, v in inputs.items() if k != "edge_index"})
    nc = build_program(s)
    res = run_bass_kernel_spmd(nc, in_maps, core_ids=list(range(d["cores"])),
                               trace=trace)
    outs = [np.asarray(res.results[c]["out"], np.float32).T
            for c in range(d["cores"])]
    out_full = np.concatenate(outs, axis=0)  # [nptot, 2]
    final = out_full[s.perm]                 # original node order
    return final.astype(np.float32), res


def kernel(**inputs):
    out, _ = run(FULL_CFG, inputs)
    return out



# revision 59
# speedup vs baseline: 1.0199x; 1.0199x over previous
"""Trainium2 Bass kernel for a 3-type heterogeneous GraphSAGE GNN.

Full-input contract: kernel(**inputs) takes the unsharded numpy inputs and
returns the full [300000, 2] output. Internally (SPMD over 8 cores):
  - Nodes are relabeled so each core owns a contiguous local range of
    37632 padded nodes (12544 per type); edges are sharded by dst owner.
    The replicated x_full (xf) uses a piece-major global numbering so each
    AllGather piece lands contiguously and gates only the src blocks it
    covers.
  - xf / x_own store compact bf16 features as node-PAIR rows
    [n_nodes/2, 128]: node n at row n//2, col half (n%2)*64. This halves
    AllGather bytes vs f32 while keeping the 256B gather-element
    granularity the SWDGE path requires; each gather descriptor fetches
    one pair row and per-chunk even/odd-parity one-hots route the right
    half into the aggregation matmuls.
  - Per core, edges are sorted by (pass, src block of 32768, dst group,
    src) and padded into a uniform schedule of 128-edge chunks (one NEFF,
    per-core data). gpsimd.dma_gather pulls pair rows in 1024-index calls
    over 4 SWDGE queues; idx/dstrel DMAs are batched in 64-chunk strips.
  - Segment-sum: bf16 one-hots [128 edges x 128 dsts] built on DVE via
    is_equal(dstrel_even/odd, iota) feed two accumulating PE matmuls per
    chunk (even half, odd half of the gathered pair) into a 64-slot f32
    PSUM ring; banks flush into the SBUF accumulator agg; mean = agg *
    winv per post group. bf16 operands keep PE at 1 cycle/row and avoid
    the f32 2-pass LDWEIGHTS split.
  - The encoder consumes a host-transposed xrawT [49, npc] bf16 (ones row
    folds the bias); the x@W1r term of layer 1 is folded into the encoder
    weights, and y1@W2r + b2 is computed during post-L1 (xwr buffer).
    The classifier runs transposed and emits out^T [2, npc]; the host
    untransposes and un-permutes.
  - A per-layer probe DMA + Pool gate op data-depends on the layer's
    AllGather pieces before the first gather of that layer.
"""

import numpy as np
import ml_dtypes

import bass_rust
import concourse.bass as bass
import concourse.bacc as bacc
import concourse.mybir as mybir
import concourse.tile as tile
from concourse.masks import make_identity
from concourse.bass_utils import run_bass_kernel_spmd

F32 = mybir.dt.float32
BF16 = mybir.dt.bfloat16
I16 = mybir.dt.int16
BF = ml_dtypes.bfloat16

FULL_CFG = dict(type_size=100000, E=4800000, cores=8, blk=65536,
                strip_chunks=64, oh_chunks=16, gather_chunks=8,
                ag_pieces=3, layer_splits=2, split_tail_blocks=2,
                swdge_queues=4, h=64)


def derive(cfg):
    cores = cfg["cores"]
    seg = cfg["type_size"] // cores          # real nodes per (core, type)
    assert seg * cores == cfg["type_size"]
    segp = -(-seg // 128) * 128              # padded to tile multiple
    npc = 3 * segp                           # nodes per core (padded)
    nptot = cores * npc
    tiles = npc // 128
    groups = tiles                           # 128-dst groups per core
    nblk = -(-nptot // cfg["blk"])
    d = dict(cfg)
    d.update(seg=seg, segp=segp, npc=npc, nptot=nptot, tiles=tiles,
             groups=groups, nblk=nblk)
    return d


def node_perm(d):
    """perm_of_orig[j] = per-core-local padded id of original node j
    (core-major: core * npc + t * segp + local)."""
    ts, seg, segp, npc = d["type_size"], d["seg"], d["segp"], d["npc"]
    j = np.arange(ts)
    core = j // seg
    local = j % seg
    parts = [core * npc + t * segp + local for t in range(3)]
    return np.concatenate(parts)


def glob_of_loc(d):
    """Map core-major local id -> piece-major xf row (AllGather pieces:
    xf = [piece s: core0 slice s, core1 slice s, ...])."""
    npc, nptot, cores = d["npc"], d["nptot"], d["cores"]
    P = d["ag_pieces"]
    psz = npc // P
    assert psz * P == npc
    lid = np.arange(nptot)
    c = lid // npc
    pos = lid % npc
    s = pos // psz
    return s * (cores * psz) + c * psz + pos % psz


class Sched:
    pass


def plan(d, edge_index):
    """Build the uniform schedule + per-core edge data arrays."""
    cores, npc, nptot, blk = d["cores"], d["npc"], d["nptot"], d["blk"]
    groups, nblk, sc = d["groups"], d["nblk"], d["strip_chunks"]
    ohc = d["oh_chunks"]
    gc = d["gather_chunks"]
    nsp = d["layer_splits"]
    gps = groups // nsp
    assert gps * nsp == groups
    FB = nblk - d["split_tail_blocks"]

    perm = node_perm(d)
    g_of_l = glob_of_loc(d)
    src_p = g_of_l[perm[np.asarray(edge_index[0], dtype=np.int64)]]
    dst_p = perm[np.asarray(edge_index[1], dtype=np.int64)]

    deg = np.bincount(dst_p, minlength=nptot).astype(np.float64)
    winv_full = (1.0 / np.maximum(deg, 1.0)).astype(np.float32)

    # per-core sorted edge arrays + per-(block, group) counts
    core_of = dst_p // npc
    per_core = []
    counts = np.zeros((cores, nblk, groups), np.int64)
    for c in range(cores):
        m = core_of == c
        es = src_p[m]
        ed = dst_p[m] - c * npc
        b = es // blk
        g = ed // 128
        # cells contiguous by pass; within cell sort by src
        pr = np.where(b < FB, b, FB + (g // gps) * (nblk - FB) + (b - FB))
        order = np.lexsort((es, g, pr))
        es, ed, b, g = es[order], ed[order], b[order], g[order]
        np.add.at(counts[c], (b, g), 1)
        per_core.append((es, ed, b, g))

    nch = np.maximum(1, -(-counts.max(axis=0) // 128))  # [nblk, groups]

    # cell order: blocks 0..FB-1 run all groups (block-major, matching
    # AllGather piece arrival); the last split_tail_blocks blocks run in
    # group-split passes so the first gps groups of agg complete early,
    # letting post + the next AllGather piece overlap the layer tail
    passes = [(b, 0, groups) for b in range(FB)]
    for sp in range(nsp):
        for b in range(FB, nblk):
            passes.append((b, sp * gps, (sp + 1) * gps))
    cell_b_arr, cell_g_arr, cell_g0 = [], [], []
    for (b, g0, g1) in passes:
        for g in range(g0, g1):
            cell_b_arr.append(b)
            cell_g_arr.append(g)
            cell_g0.append(g0)
    cell_b_arr = np.array(cell_b_arr)
    cell_g_arr = np.array(cell_g_arr)
    cell_g0 = np.array(cell_g0)
    cell_pass = np.concatenate(
        [np.full(g1 - g0, pi) for pi, (b, g0, g1) in enumerate(passes)])
    cell_g1 = np.array([passes[p][2] for p in cell_pass])
    nch_ord = nch[cell_b_arr, cell_g_arr]

    chunk_cell = np.repeat(np.arange(len(cell_b_arr)), nch_ord)
    chunk_b = cell_b_arr[chunk_cell]
    chunk_g = cell_g_arr[chunk_cell]
    chunk_g0 = cell_g0[chunk_cell]
    chunk_g1 = cell_g1[chunk_cell]
    chunk_pass = cell_pass[chunk_cell]
    nchunks = len(chunk_b)
    first_of_cell = np.concatenate([[0], np.cumsum(nch_ord)])[:-1]
    kk = np.arange(nchunks) - first_of_cell[chunk_cell]
    chunk_start = kk == 0
    chunk_stop = kk == nch_ord[chunk_cell] - 1

    # strips: cut chunk list per pass at strip_chunks boundary
    strips = []  # (b, c0, n, idx_col_off)
    idx_off = 0
    i = 0
    while i < nchunks:
        j = i
        while (j < nchunks and chunk_pass[j] == chunk_pass[i]
               and j - i < sc):
            j += 1
        strips.append((int(chunk_b[i]), i, j - i, idx_off))
        idx_off += (j - i) * 8
        i = j
    idx_cols = idx_off
    strip_of_chunk = np.zeros(nchunks, np.int64)
    strip_c0 = np.zeros(nchunks, np.int64)
    for si, (b, c0, n, io) in enumerate(strips):
        strip_of_chunk[c0:c0 + n] = si
        strip_c0[c0:c0 + n] = c0

    # op lists (two parts: h0 = full blocks + first tail split; h1 = rest)
    npass_h0 = FB + (nblk - FB)
    ops_parts = [[], []]
    for si, (b, c0, n, io) in enumerate(strips):
        ops = ops_parts[0 if chunk_pass[c0] < npass_h0 else 1]
        ops.append(("strip", si))
        for k in range(n):
            if k % gc == 0:
                ops.append(("gather", si, k, min(gc, n - k)))
            if k % ohc == 0:
                ops.append(("oh", si, k, min(ohc, n - k)))
            ci = c0 + k
            g = int(chunk_g[ci])
            g0 = int(chunk_g0[ci])
            g_rel = g - g0
            ops.append(("mm", si, k, g_rel,
                        bool(chunk_start[ci]), bool(chunk_stop[ci])))
            nxt = ci + 1
            if chunk_stop[ci]:
                bank_end = (nxt == nchunks
                            or chunk_pass[nxt] != chunk_pass[ci]
                            or (chunk_g[nxt] - g0) // 8 != g_rel // 8)
                if bank_end:
                    r_lo = (g_rel // 8) * 8
                    g_lo = g0 + r_lo
                    g_hi = min(g_lo + 7, int(chunk_g1[ci]) - 1)
                    ops.append(("flush", g_lo, g_hi, r_lo))

    # ---- per-core data arrays ----
    # padded slots: each ordered cell occupies nch*128 consecutive slots
    cell_nslots = nch_ord * 128
    cell_pad_start = np.concatenate([[0], np.cumsum(cell_nslots)])
    total_slots = int(cell_pad_start[-1])
    assert total_slots == nchunks * 128
    slot = np.arange(total_slots)
    cell_of_slot = np.searchsorted(cell_pad_start, slot, "right") - 1
    within = slot - cell_pad_start[cell_of_slot]

    # chunk index of each slot & strip-local edge index
    ch_of_slot = slot // 128
    strip_local = (ch_of_slot - strip_c0[ch_of_slot]) * 128 + slot % 128
    idx_col = np.array([strips[s][3] for s in strip_of_chunk[ch_of_slot]]) \
        + strip_local // 16
    idx_row = strip_local % 16
    slot_b = chunk_b[ch_of_slot]
    slot_g = chunk_g[ch_of_slot]

    # per-chunk even/odd dstrel column layout: strip si (chunks c0..c0+n)
    # occupies dstrel cols [2*c0, 2*c0+2n): even-parity block then odd.
    ch_ids = np.arange(nchunks)
    col_e = 2 * strip_c0[ch_ids] + (ch_ids - strip_c0[ch_ids])
    col_o = col_e + np.array([strips[s_][2] for s_ in strip_of_chunk])

    idx_all = np.zeros((cores, 128, idx_cols), np.int16)
    dstrel_all = np.full((cores, 128, 2 * nchunks), -1.0, np.float32)
    for c in range(cores):
        es, ed, b, g = per_core[c]
        ccounts = counts[c][cell_b_arr, cell_g_arr]
        cell_start = np.concatenate([[0], np.cumsum(ccounts)])
        real = within < ccounts[cell_of_slot]
        src_idx = cell_start[cell_of_slot] + np.minimum(
            within, np.maximum(ccounts[cell_of_slot] - 1, 0))
        esv = np.where(real, es[np.minimum(src_idx, len(es) - 1)]
                       if len(es) else 0, 0)
        edv = np.where(real, ed[np.minimum(src_idx, len(ed) - 1)]
                       if len(ed) else 0, -1)
        rel = np.where(real, esv - slot_b * blk, 0).astype(np.int64)
        assert rel.min() >= 0 and rel.max() < blk
        drel = np.where(real, edv - slot_g * 128, -1.0).astype(np.float32)
        # pair-row index (each 256B gather elem covers nodes 2j, 2j+1)
        for r in range(8):
            idx_all[c, idx_row + 16 * r, idx_col] = (rel >> 1).astype(np.int16)
        par = (esv % 2).astype(np.int64)
        cols = np.where(par == 0, col_e[ch_of_slot], col_o[ch_of_slot])
        dstrel_all[c, slot % 128, cols] = drel

    s = Sched()
    s.d = d
    s.perm = perm
    s.strips = strips
    s.ops_parts = ops_parts
    s.gps = gps
    s.nchunks = nchunks
    s.idx_cols = idx_cols
    s.winv_full = winv_full
    s.idx_all = idx_all
    s.dstrel_all = dstrel_all
    return s


def core_inputs(s, x_individual, x_company, x_trust,
                W_ind, b_ind, W_com, b_com, W_tru, b_tru,
                W1l, W1r, b1, W2l, W2r, b2, Wc1, bc1, Wc2, bc2):
    d = s.d
    cores, seg, segp, npc, groups = \
        d["cores"], d["seg"], d["segp"], d["npc"], d["groups"]
    raws = [np.asarray(x_individual, np.float32),
            np.asarray(x_company, np.float32),
            np.asarray(x_trust, np.float32)]
    Ws = [np.asarray(W_ind, np.float32), np.asarray(W_com, np.float32),
          np.asarray(W_tru, np.float32)]
    bs = [np.asarray(b_ind, np.float32), np.asarray(b_com, np.float32),
          np.asarray(b_tru, np.float32)]
    h = d["h"]
    kenc = 49  # 48 padded features + ones row

    W1l = np.asarray(W1l, np.float32)
    W1r = np.asarray(W1r, np.float32)
    b1 = np.asarray(b1, np.float32)
    W2l = np.asarray(W2l, np.float32)
    W2r = np.asarray(W2r, np.float32)
    b2 = np.asarray(b2, np.float32)
    Wc1 = np.asarray(Wc1, np.float32)
    bc1 = np.asarray(bc1, np.float32)
    Wc2 = np.asarray(Wc2, np.float32)
    bc2 = np.asarray(bc2, np.float32)

    shared = {}
    for t in range(3):
        wx = np.zeros((kenc, h), np.float32)
        wx[:Ws[t].shape[0], :] = Ws[t]
        wx[48, :] = bs[t]
        shared[f"w_enc_x{t}"] = wx.astype(BF)
        # folded x@W1r path: (x_enc) @ W1r + b1, from raw features
        wr = np.zeros((kenc, h), np.float32)
        wr[:Ws[t].shape[0], :] = Ws[t] @ W1r
        wr[48, :] = bs[t] @ W1r + b1
        shared[f"w_enc_r{t}"] = wr.astype(BF)

    w2r_ext = np.zeros((h + 1, h), np.float32)
    w2r_ext[:h, :] = W2r
    w2r_ext[h, :] = b2
    wc1_ext = np.zeros((h + 1, 32), np.float32)
    wc1_ext[:h, :] = Wc1
    wc1_ext[h, :] = bc1
    wc2_ext = np.zeros((33, 2), np.float32)
    wc2_ext[:32, :] = Wc2
    wc2_ext[32, :] = bc2

    shared.update({
        "w1l": W1l.astype(BF), "w2l": W2l.astype(BF),
        "w2r_ext": w2r_ext.astype(BF),
        "wc1_ext": wc1_ext.astype(BF), "wc2_ext": wc2_ext.astype(BF),
        "iota_rep": np.tile(np.arange(128, dtype=np.float32)[None, :],
                            (128, 1)).astype(BF),
    })

    in_maps = []
    for c in range(cores):
        xrT = np.zeros((kenc, npc), np.float32)
        for t in range(3):
            r0 = t * segp
            xrT[:raws[t].shape[1], r0:r0 + seg] = \
                raws[t][c * seg:(c + 1) * seg].T
            xrT[48, r0:r0 + seg] = 1.0
        winv = s.winv_full[c * npc:(c + 1) * npc] \
            .reshape(groups, 128).T.copy()
        m = dict(shared)
        m.update(xrawT=xrT.astype(BF), idx=s.idx_all[c],
                 dstrel=s.dstrel_all[c].astype(BF), winv=winv)
        in_maps.append(m)
    return in_maps


def build_program(s, skip=()):
    skip = set(skip)
    d = s.d
    cores, npc, nptot, blk = d["cores"], d["npc"], d["nptot"], d["blk"]
    tiles, groups, nblk, h = d["tiles"], d["groups"], d["nblk"], d["h"]
    sc, ohc = d["strip_chunks"], d["oh_chunks"]
    slots = 64
    kenc = 49
    GT = 8                                   # tiles per post/encoder group
    ngroups = -(-tiles // GT)
    seg_tiles = d["segp"] // 128

    nc = bacc.Bacc("TRN2", target_bir_lowering=False, debug=False,
                   num_devices=cores,
                   num_swdge_queues=d.get("swdge_queues", 1),
                   dynamic_dma_scratch_size=d.get("dma_scratch", 16384))

    di = {}
    def inp(name, shape, dt=F32):
        di[name] = nc.dram_tensor(name, list(shape), dt, kind="ExternalInput")
        return di[name]

    inp("xrawT", [kenc, npc], BF16)
    if "nocoll" in skip:
        inp("xf0_in", [nptot // 2, 2 * h], BF16)
        inp("xf1_in", [nptot // 2, 2 * h], BF16)
    inp("idx", [128, s.idx_cols], I16)
    inp("dstrel", [128, 2 * s.nchunks], BF16)
    inp("winv", [128, groups])
    inp("iota_rep", [128, 128], BF16)
    for t in range(3):
        inp(f"w_enc_x{t}", [kenc, h], BF16)
        inp(f"w_enc_r{t}", [kenc, h], BF16)
    inp("w1l", [h, h], BF16)
    inp("w2l", [h, h], BF16)
    inp("w2r_ext", [h + 1, h], BF16)
    inp("wc1_ext", [h + 1, 32], BF16)
    inp("wc2_ext", [33, 2], BF16)
    out_d = nc.dram_tensor("out", [2, npc], F32, kind="ExternalOutput")

    AG = "AllGather"
    ADD = mybir.AluOpType.add
    MUL = mybir.AluOpType.mult
    EQ = mybir.AluOpType.is_equal
    BYP = mybir.AluOpType.bypass
    RELU = mybir.ActivationFunctionType.Relu

    with tile.TileContext(nc) as tc:
        with tc.tile_pool(name="persist", bufs=1) as pp, \
             tc.tile_pool(name="dram", bufs=1, space="DRAM") as dramp:
            # constants to SBUF
            def csb(name, shape, dt=F32):
                t_ = pp.tile(list(shape), dt, tag=name)
                nc.sync.dma_start(t_[:], di[name].ap())
                return t_
            iota_sb = csb("iota_rep", [128, 128], BF16)
            winv_sb = csb("winv", [128, groups])
            wex_sb = [csb(f"w_enc_x{t}", [kenc, h], BF16) for t in range(3)]
            wer_sb = [csb(f"w_enc_r{t}", [kenc, h], BF16) for t in range(3)]
            w1l_sb = csb("w1l", [h, h], BF16)
            w2l_sb = csb("w2l", [h, h], BF16)
            w2r_sb = csb("w2r_ext", [h + 1, h], BF16)
            wc1_sb = csb("wc1_ext", [h + 1, 32], BF16)
            wc2_sb = csb("wc2_ext", [33, 2], BF16)
            ident = pp.tile([128, 128], F32, tag="ident")
            make_identity(nc, ident[:])
            identb = pp.tile([128, 128], BF16, tag="identb")
            nc.scalar.copy(identb[:], ident[:])

            # AG-completion sems: gathers of layer L wait for all of layer
            # L's AllGather pieces before issuing, so collective traffic
            # never contends with gather descriptor streams. Cleared at
            # start (sems persist across NEFF re-runs).
            agp_sb = [pp.tile([1, 2 * h], BF16, tag="agp0", name="agp0"),
                      pp.tile([1, 2 * h], BF16, tag="agp1", name="agp1")]
            gate_sb = pp.tile([1, 2 * h], BF16, tag="gate")

            def ag_probe(L, end_row):
                # 1-row read of the piece tail; Tile orders it after the
                # collective's output write.
                nc.sync.dma_start(agp_sb[L][:],
                                  xf[L][end_row - 1:end_row, :])

            agg = pp.tile([128, groups * h], F32, tag="agg")
            xwr = pp.tile([128, groups * h], BF16, tag="xwr")

            # x_own/xf hold compact bf16 features as node-PAIR rows
            # [n_nodes/2, 2h]: node n lives at row n//2, col half (n%2)*h.
            # Each 256B gather elem fetches one pair row; per chunk the
            # even/odd-parity one-hots route the right half into agg.
            x_own0 = dramp.tile([npc // 2, 2 * h], BF16)
            x_own1 = dramp.tile([npc // 2, 2 * h], BF16)
            xf = [dramp.tile([nptot // 2, 2 * h], BF16, name="xf0"),
                  dramp.tile([nptot // 2, 2 * h], BF16, name="xf1")]
            x_own = [x_own0, x_own1]
            if "nocoll" in skip:
                xf_slice = lambda L, a, b: \
                    di[f"xf{L}_in"].ap()[a // 2:b // 2, :]
            else:
                xf_slice = lambda L, a, b: xf[L][a // 2:b // 2, :]

            def pair_write_ap(own, t0, gt):
                # dest AP for nodes [t0*128, (t0+gt)*128) in pair-row
                # layout; flat elem offset of node n is n*h, so the
                # (p, t, f) iteration is affine: [[h,128],[128*h,gt],[1,h]].
                a = own[:, :].copy()
                a.ap = bass_rust.VecI64Pair([[h, 128], [128 * h, gt], [1, h]])
                a.offset = t0 * 128 * h
                return a

            # ---------------- encoder ----------------
            with tc.tile_pool(name="encio", bufs=1) as pio, \
                 tc.tile_pool(name="enc", bufs=2) as pe, \
                 tc.tile_pool(name="encps", bufs=2, space="PSUM") as pse:
                xrT = pio.tile([kenc, tiles * 128], BF16)
                nc.sync.dma_start(xrT[:], di["xrawT"].ap())
                GT_E = 16
                for gi in range(0 if "enc" in skip else -(-tiles // GT_E)):
                    t0 = gi * GT_E
                    gt = min(GT_E, tiles - t0)
                    ps_x = pse.tile([128, GT_E * h], F32, tag="psx")
                    ps_w = pse.tile([128, GT_E * h], F32, tag="psw")
                    for k in range(gt):
                        t = t0 + k
                        ty = t // seg_tiles
                        lhs = xrT[:, t * 128:(t + 1) * 128]
                        nc.tensor.matmul(out=ps_x[:, k * h:(k + 1) * h],
                                         lhsT=lhs, rhs=wex_sb[ty][:],
                                         start=True, stop=True)
                        nc.tensor.matmul(out=ps_w[:, k * h:(k + 1) * h],
                                         lhsT=lhs, rhs=wer_sb[ty][:],
                                         start=True, stop=True)
                    x0s = pe.tile([128, GT_E * h], BF16, tag="x0s")
                    nc.scalar.copy(x0s[:, :gt * h], ps_x[:, :gt * h])
                    nc.sync.dma_start(
                        pair_write_ap(x_own0, t0, gt),
                        x0s[:, :gt * h].rearrange("p (t f) -> p t f", f=h))
                    nc.scalar.copy(xwr[:, t0 * h:(t0 + gt) * h],
                                   ps_w[:, :gt * h])
            P0 = d["ag_pieces"]
            psz0 = npc // P0
            for sp in range(P0 if "nocoll" not in skip else 0):
                nc.gpsimd.collective_compute(
                    AG, BYP, replica_groups=[list(range(cores))],
                    ins=[x_own0[sp * psz0 // 2:(sp + 1) * psz0 // 2, :]],
                    outs=[xf[0][sp * cores * psz0 // 2:
                                (sp + 1) * cores * psz0 // 2, :]])
                ag_probe(0, (sp + 1) * cores * psz0 // 2)

            # -------- SAGE layers (post/cls interleaved at h0/h1) -------
            GT_C = 8
            ngroups_c = -(-tiles // GT_C)
            gps = s.gps
            gi_h0 = gps // GT          # post groups fully inside h0
            gi_h0_c = gps // GT_C
            P = d["ag_pieces"]
            psz = npc // P

            def emit_ops(ops, pa, po, psum_agg, L, slots, wait=None):
                if "sage" in skip:
                    return
                gate = wait if "nocoll" not in skip else None
                cur = {}
                ohmod = d.get("oh_pool_mod", 0)
                ohi = 0
                for op in ops:
                    if op[0] == "strip":
                        si = op[1]
                        b, c0, n, ioff = s.strips[si]
                        idx_sb = pa.tile([128, sc * 8], I16, tag="idx")
                        nc.sync.dma_start(
                            idx_sb[:, :n * 8],
                            di["idx"].ap()[:, ioff:ioff + n * 8])
                        dst_sb = pa.tile([128, 2 * sc], BF16, tag="dst")
                        nc.sync.dma_start(
                            dst_sb[:, :2 * n],
                            di["dstrel"].ap()[:, 2 * c0:2 * c0 + 2 * n])
                        msgs = pa.tile([128, sc * 2 * h], BF16, tag="msgs")
                        if gate is not None:
                            # Pool op whose output the first gather
                            # overwrites: WAR-chains all gathers behind this
                            # layer's AllGather pieces (via the probe tile).
                            nc.gpsimd.tensor_tensor(
                                out=msgs[0:1, 0:2 * h],
                                in0=agp_sb[gate][:],
                                in1=agp_sb[gate][:], op=ADD)
                            gate = None
                        rows = min(blk, nptot - b * blk)
                        cur = dict(msgs=msgs, dst=dst_sb, idx=idx_sb,
                                   b=b, rows=rows, n=n)
                    elif op[0] == "gather":
                        _, si, k0, m = op
                        b = cur["b"]
                        rows = cur["rows"]
                        nq = d.get("swdge_queues", 1)
                        nc.gpsimd.dma_gather(
                            out_ap=cur["msgs"][:, k0 * 2 * h:(k0 + m) * 2 * h]
                            .rearrange("p (c f) -> p c f", f=2 * h),
                            in_ap=xf_slice(L, b * blk, b * blk + rows),
                            idxs_ap=cur["idx"][:, k0 * 8:(k0 + m) * 8],
                            num_idxs=m * 128, num_idxs_reg=m * 128,
                            elem_size=2 * h,
                            queue_num=(k0 // 8) % nq)
                    elif op[0] == "oh":
                        _, si, k0, m = op
                        n = cur["n"]
                        oh = po.tile([128, 2 * ohc * 128], BF16, tag="oh")
                        ohi += 1
                        eng = (nc.gpsimd if ohmod and ohi % ohmod == 0
                               else nc.vector)
                        for half, cofs in ((0, k0), (1, n + k0)):
                            eng.tensor_tensor(
                                out=oh[:, half * ohc * 128:
                                       half * ohc * 128 + m * 128]
                                .rearrange("p (c w) -> p c w", w=128),
                                in0=cur["dst"][:, cofs:cofs + m][:, :, None]
                                .to_broadcast([128, m, 128]),
                                in1=iota_sb[:][:, None, :]
                                .to_broadcast([128, m, 128]),
                                op=EQ)
                        cur["oh"] = oh
                        cur["k0"] = k0
                    elif op[0] == "mm":
                        _, si, k, g_rel, st, sp = op
                        sl = g_rel % slots
                        ko = k - cur["k0"]
                        nc.tensor.matmul(
                            out=psum_agg[:, sl * h:(sl + 1) * h],
                            lhsT=cur["oh"][:, ko * 128:(ko + 1) * 128],
                            rhs=cur["msgs"][:, k * 2 * h:k * 2 * h + h],
                            start=st, stop=False)
                        nc.tensor.matmul(
                            out=psum_agg[:, sl * h:(sl + 1) * h],
                            lhsT=cur["oh"][:, ohc * 128 + ko * 128:
                                           ohc * 128 + (ko + 1) * 128],
                            rhs=cur["msgs"][:, k * 2 * h + h:
                                            (k + 1) * 2 * h],
                            start=False, stop=sp)
                    else:  # flush
                        _, g_lo, g_hi, r_lo = op
                        sl = r_lo % slots
                        w = (g_hi - g_lo + 1) * h
                        nc.vector.tensor_tensor(
                            out=agg[:, g_lo * h:g_lo * h + w],
                            in0=agg[:, g_lo * h:g_lo * h + w],
                            in1=psum_agg[:, sl * h:sl * h + w],
                            op=ADD)

            def mean_groups(gi, t0, gt):
                nc.vector.tensor_tensor(
                    out=agg[:, t0 * h:(t0 + gt) * h].rearrange(
                        "p (g f) -> p g f", f=h),
                    in0=agg[:, t0 * h:(t0 + gt) * h].rearrange(
                        "p (g f) -> p g f", f=h),
                    in1=winv_sb[:, t0:t0 + gt][:, :, None]
                    .to_broadcast([128, gt, h]),
                    op=MUL)

            def post_l1(pq, psp, gi0, gi1):
                if "post1" in skip:
                    return
                # y1 = relu(mean@W1l + xwr1); xwr2 = y1@W2r + b2
                for gi in range(gi0, gi1):
                    t0 = gi * GT
                    gt = min(GT, tiles - t0)
                    mean_groups(gi, t0, gt)
                    mb = pq.tile([128, GT * h], BF16, tag="mb")
                    nc.scalar.copy(mb[:, :gt * h],
                                   agg[:, t0 * h:(t0 + gt) * h])
                    tp = psp.tile([64, GT * 128], BF16, tag="tp")
                    for k in range(gt):
                        nc.tensor.transpose(
                            out=tp[:, k * 128:(k + 1) * 128],
                            in_=mb[:, k * h:(k + 1) * h],
                            identity=identb[:])
                    mT = pq.tile([64, GT * 128], BF16, tag="mT")
                    nc.scalar.copy(mT[:, :gt * 128], tp[:, :gt * 128])
                    ym = psp.tile([128, GT * h], F32, tag="ym")
                    for k in range(gt):
                        nc.tensor.matmul(
                            out=ym[:, k * h:(k + 1) * h],
                            lhsT=mT[:, k * 128:(k + 1) * 128],
                            rhs=w1l_sb[:], start=True, stop=True)
                    tmp = pq.tile([128, GT * h], F32, tag="tmp")
                    nc.vector.tensor_tensor(
                        out=tmp[:, :gt * h], in0=ym[:, :gt * h],
                        in1=xwr[:, t0 * h:(t0 + gt) * h], op=ADD)
                    ys = pq.tile([128, GT * h], F32, tag="ys")
                    nc.scalar.activation(out=ys[:, :gt * h],
                                         in_=tmp[:, :gt * h], func=RELU)
                    yb = pq.tile([128, GT * h], BF16, tag="yb")
                    nc.scalar.copy(yb[:, :gt * h], ys[:, :gt * h])
                    nc.sync.dma_start(
                        pair_write_ap(x_own1, t0, gt),
                        yb[:, :gt * h].rearrange("p (t f) -> p t f", f=h))
                    tp2 = psp.tile([64, GT * 128], BF16, tag="tp")
                    for k in range(gt):
                        nc.tensor.transpose(
                            out=tp2[:, k * 128:(k + 1) * 128],
                            in_=yb[:, k * h:(k + 1) * h],
                            identity=identb[:])
                    yT = pq.tile([h + 1, GT * 128], BF16, tag="yT")
                    nc.scalar.copy(yT[:h, :gt * 128], tp2[:, :gt * 128])
                    nc.vector.memset(yT[h:h + 1, :gt * 128], 1.0)
                    xw2 = psp.tile([128, GT * h], F32, tag="ym")
                    for k in range(gt):
                        nc.tensor.matmul(
                            out=xw2[:, k * h:(k + 1) * h],
                            lhsT=yT[:, k * 128:(k + 1) * 128],
                            rhs=w2r_sb[:], start=True, stop=True)
                    nc.scalar.copy(xwr[:, t0 * h:(t0 + gt) * h],
                                   xw2[:, :gt * h])

            def cls_range(pc, psc, gi0, gi1):
                if "cls" in skip:
                    return
                # y2 = relu(mean@W2l + xwr2); h = relu(Wc1^T y2T);
                # outT = Wc2^T hT  (biases folded via ones partitions)
                for gi in range(gi0, gi1):
                    t0 = gi * GT_C
                    gt = min(GT_C, tiles - t0)
                    mean_groups(gi, t0, gt)
                    mb = pc.tile([128, GT_C * h], BF16, tag="mb")
                    nc.scalar.copy(mb[:, :gt * h],
                                   agg[:, t0 * h:(t0 + gt) * h])
                    tp = psc.tile([64, GT_C * 128], BF16, tag="tp")
                    for k in range(gt):
                        nc.tensor.transpose(
                            out=tp[:, k * 128:(k + 1) * 128],
                            in_=mb[:, k * h:(k + 1) * h],
                            identity=identb[:])
                    mT = pc.tile([64, GT_C * 128], BF16, tag="mT")
                    nc.scalar.copy(mT[:, :gt * 128], tp[:, :gt * 128])
                    ym = psc.tile([128, GT_C * h], F32, tag="ym")
                    for k in range(gt):
                        nc.tensor.matmul(
                            out=ym[:, k * h:(k + 1) * h],
                            lhsT=mT[:, k * 128:(k + 1) * 128],
                            rhs=w2l_sb[:], start=True, stop=True)
                    tmp = pc.tile([128, GT_C * h], F32, tag="tmp")
                    nc.vector.tensor_tensor(
                        out=tmp[:, :gt * h], in0=ym[:, :gt * h],
                        in1=xwr[:, t0 * h:(t0 + gt) * h], op=ADD)
                    y2 = pc.tile([128, GT_C * h], BF16, tag="y2")
                    nc.scalar.activation(out=y2[:, :gt * h],
                                         in_=tmp[:, :gt * h], func=RELU)
                    tpb = psc.tile([64, GT_C * 128], BF16, tag="tp")
                    for k in range(gt):
                        nc.tensor.transpose(
                            out=tpb[:, k * 128:(k + 1) * 128],
                            in_=y2[:, k * h:(k + 1) * h],
                            identity=identb[:])
                    y2T = pc.tile([h + 1, GT_C * 128], BF16, tag="y2T")
                    nc.scalar.copy(y2T[:h, :gt * 128], tpb[:, :gt * 128])
                    nc.vector.memset(y2T[h:h + 1, :gt * 128], 1.0)
                    for half in range(2):
                        k0 = half * (GT_C // 2)
                        k1 = min(k0 + GT_C // 2, gt)
                        if k0 >= gt:
                            break
                        kw = k1 - k0
                        hps = psc.tile([32, (GT_C // 2) * 128], F32,
                                       tag="hps")
                        for k in range(k0, k1):
                            nc.tensor.matmul(
                                out=hps[:, (k - k0) * 128:
                                        (k - k0 + 1) * 128],
                                lhsT=wc1_sb[:],
                                rhs=y2T[:, k * 128:(k + 1) * 128],
                                start=True, stop=True)
                        hT = pc.tile([33, (GT_C // 2) * 128], BF16,
                                     tag="hT")
                        nc.scalar.activation(out=hT[:32, :kw * 128],
                                             in_=hps[:, :kw * 128],
                                             func=RELU)
                        nc.vector.memset(hT[32:33, :kw * 128], 1.0)
                        ops_ = psc.tile([2, (GT_C // 2) * 128], F32,
                                        tag="ops")
                        for k in range(kw):
                            nc.tensor.matmul(
                                out=ops_[:, k * 128:(k + 1) * 128],
                                lhsT=wc2_sb[:],
                                rhs=hT[:, k * 128:(k + 1) * 128],
                                start=True, stop=True)
                        outs = pc.tile([2, (GT_C // 2) * 128], F32,
                                       tag="outs")
                        nc.scalar.copy(outs[:, :kw * 128],
                                       ops_[:, :kw * 128])
                        nc.sync.dma_start(
                            out_d.ap()[:, (t0 + k0) * 128:
                                       (t0 + k1) * 128],
                            outs[:, :kw * 128])

            # ---- layer 1 ----
            with tc.tile_pool(name="sage0", bufs=3) as pa, \
                 tc.tile_pool(name="sageoh0", bufs=2) as po, \
                 tc.tile_pool(name="post0", bufs=2) as pq:
                nc.vector.memset(agg[:], 0.0)
                with tc.tile_pool(name="sageps0a", bufs=1,
                                  space="PSUM") as psa:
                    psum_agg = psa.tile([128, 64 * h], F32)
                    emit_ops(s.ops_parts[0], pa, po, psum_agg, 0, 64,
                             wait=0)
                gi_ag = -(-(psz // 128) // GT)
                with tc.tile_pool(name="post0psa", bufs=2,
                                  space="PSUM") as psp:
                    post_l1(pq, psp, 0, gi_ag)
                    if "nocoll" not in skip:
                        nc.gpsimd.collective_compute(
                            AG, BYP, replica_groups=[list(range(cores))],
                            ins=[x_own1[0:psz // 2, :]],
                            outs=[xf[1][0:cores * psz // 2, :]])
                        ag_probe(1, cores * psz // 2)
                    post_l1(pq, psp, gi_ag, gi_h0)
                with tc.tile_pool(name="sageps0b", bufs=1,
                                  space="PSUM") as psa:
                    psum_agg = psa.tile([128, 64 * h], F32)
                    emit_ops(s.ops_parts[1], pa, po, psum_agg, 0, 64)
                with tc.tile_pool(name="post0psb", bufs=2,
                                  space="PSUM") as psp:
                    post_l1(pq, psp, gi_h0, ngroups)
                for sp in range(1, P if "nocoll" not in skip else 1):
                    nc.gpsimd.collective_compute(
                        AG, BYP, replica_groups=[list(range(cores))],
                        ins=[x_own1[sp * psz // 2:(sp + 1) * psz // 2, :]],
                        outs=[xf[1][sp * cores * psz // 2:
                                    (sp + 1) * cores * psz // 2, :]])
                    ag_probe(1, (sp + 1) * cores * psz // 2)

            # ---- layer 2 + classifier ----
            with tc.tile_pool(name="sage1", bufs=3) as pa, \
                 tc.tile_pool(name="sageoh1", bufs=2) as po, \
                 tc.tile_pool(name="cls", bufs=2) as pc:
                nc.vector.memset(agg[:], 0.0)
                with tc.tile_pool(name="sageps1a", bufs=1,
                                  space="PSUM") as psa:
                    psum_agg = psa.tile([128, 64 * h], F32)
                    emit_ops(s.ops_parts[0], pa, po, psum_agg, 1, 64,
                             wait=1)
                with tc.tile_pool(name="clspsa", bufs=2,
                                  space="PSUM") as psc:
                    cls_range(pc, psc, 0, gi_h0_c)
                with tc.tile_pool(name="sageps1b", bufs=1,
                                  space="PSUM") as psa:
                    psum_agg = psa.tile([128, 64 * h], F32)
                    emit_ops(s.ops_parts[1], pa, po, psum_agg, 1, 64)
                with tc.tile_pool(name="clspsb", bufs=2,
                                  space="PSUM") as psc:
                    cls_range(pc, psc, gi_h0_c, ngroups_c)

    nc.compile()
    return nc


def run(cfg, inputs, trace=False):
    d = derive(cfg)
    s = plan(d, inputs["edge_index"])
    in_maps = core_inputs(
        s, **{k: v for k, v in inputs.items() if k != "edge_index"})
    nc = build_program(s)
    res = run_bass_kernel_spmd(nc, in_maps, core_ids=list(range(d["cores"])),
                               trace=trace)
    outs = [np.asarray(res.results[c]["out"], np.float32).T
            for c in range(d["cores"])]
    out_full = np.concatenate(outs, axis=0)  # [nptot, 2]
    final = out_full[s.perm]                 # original node order
    return final.astype(np.float32), res


def kernel(**inputs):
    out, _ = run(FULL_CFG, inputs)
    return out

